# revision 23
# baseline (speedup 1.0000x reference)
"""Trainium2 Bass kernel for the EdgeModel GNN message-passing MLP.

Computation (per edge e):
    x = concat([src[e], dest[e], edge_attr[e], u[batch[e]]])   # [384]
    h = relu(x @ W1 + b1)                                      # [256]
    out[e] = h @ W2 + b2                                       # [64]

Sharding: data-parallel over the edge dimension E across 8 NeuronCores;
u and the MLP weights are replicated. No cross-device communication.

Device algorithm (per core, E_core = 65536 edges, tiles of 512 edges):
  - The TensorE contraction dim must live on partitions, so the x operand
    must be feature-major.  W1/W2 stay stationary in their natural
    (feature-major) layout; activations are transposed on the way in:
      * src/dest/edge_attr tiles are loaded edge-major (contiguous DMA)
        and transposed on the PE (matmul-with-identity), then copied
        PSUM -> SBUF on DVE/ACT as the layer-1 moving operands.
      * u[batch] is folded into W1: the last contraction chunk is
        [W1_ea (64 rows); u @ W1_u (16 rows)] against a rhs of
        [edge_attr^T (64); one_hot(batch) (16)].  one_hot is built with a
        DMA-replicated batch row compared against an iota column (DVE).
      * Layer 1 emits h^T (hidden-major), which is exactly the layout
        layer 2 needs; only the final [64, e] output tile needs a PE
        transpose back to edge-major before the contiguous store.
  - Default precision is fp16 transport + fp16 matmuls (fp32 PSUM
    accumulation): measured 5.9e-4 max rel err vs the fp32 reference,
    390 us HW time.  KERNEL_MM_MODE selects fp32 (exact, 1.26 ms),
    fp32r (2.2e-4, 625 us) or bf16 instead.
  - float32r matmuls need M=128 stationaries (M<128 gives garbage on HW),
    so W2 is zero-padded from 64 to 128 output columns on host (harmless
    for the other modes).
  - The DMA xbar transpose path (KERNEL_XBAR=1, off by default) is kept
    for reference but mixing xbar transposes with normal DMAs hard-crashes
    the device on this stack — do not enable.
  - DMA issue cost (~0.7 us per dma_start on the issuing engine) is
    spread across the DGE-capable queues (sync, scalar, gpsimd).
"""

import os
import sys

for _p in ("/opt/trn_rl_repo", os.path.expanduser("~/.axon_site/_ro/trn_rl_repo")):
    if os.path.isdir(_p) and _p not in sys.path:
        sys.path.insert(0, _p)

from contextlib import ExitStack

import ml_dtypes
import numpy as np

import concourse.bacc as bacc
import concourse.bass as bass
import concourse.mybir as mybir
import concourse.tile as tile
from concourse.bass_utils import run_bass_kernel_spmd
from concourse.masks import make_identity

if os.environ.get("KERNEL_LDWOPT", "0") == "1":
    # Let walrus elide/pipeline LDWEIGHTS (off by default in concourse).
    import concourse.bass_utils as _bu

    if not hasattr(_bu, "_orig_run_command"):
        _bu._orig_run_command = _bu.run_command

        def _patched_run_command(argv, **kwargs):
            argv = [
                a.replace("--enable-ldw-opt=false", "--enable-ldw-opt=true")
                for a in argv
            ]
            return _bu._orig_run_command(argv, **kwargs)

        _bu.run_command = _patched_run_command

N_CORES = 8
E_FULL = 524288
E_CORE = E_FULL // N_CORES
NODE_IN = 128
EDGE_IN = 64
GLOBAL_IN = 64
B_GLOBAL = 16
HIDDEN = 256
EDGE_OUT = 64
P = 128
TILE_E = 512
SUB = TILE_E // P  # edge sub-blocks of 128 per tile

F32 = mybir.dt.float32
F32R = mybir.dt.float32r
BF16 = mybir.dt.bfloat16
I32 = mybir.dt.int32

# "fp32": exact fp32 matmuls (slow, bit-accurate reference)
# "fp32r": f32r matmuls, fp32 transport (~2e-4 rel err)
# "fp16" (default): fp16 transport + fp16 matmuls (~5e-4 rel err, fast)
# "bf16": bf16 transport + bf16 matmuls (~4e-3 rel err, fast)
MM_MODE = os.environ.get("KERNEL_MM_MODE", "fp16")
F16 = mybir.dt.float16
MMDT = {"fp32": F32, "fp32r": F32R, "bf16": BF16, "fp16": F16}[MM_MODE]
TWO_BYTE = MM_MODE in ("bf16", "fp16")
IN_DT = MMDT if TWO_BYTE else F32
NPDT = {"fp32": np.float32, "fp32r": np.float32, "bf16": ml_dtypes.bfloat16,
        "fp16": np.float16}[MM_MODE]
XBAR = os.environ.get("KERNEL_XBAR", "0") == "1"


def build_program(e_core: int = E_CORE, num_devices: int = N_CORES):
    assert e_core % TILE_E == 0
    n_tiles = e_core // TILE_E

    nc = bacc.Bacc(
        "TRN2", target_bir_lowering=False, debug=False, num_devices=num_devices
    )

    if XBAR:
        # per-tile blocks of [hi(512 rows); lo(512 rows)] x 128 features
        srchl_d = nc.dram_tensor(
            "srchl", [2 * e_core, NODE_IN], BF16, kind="ExternalInput"
        ).ap()
        desthl_d = nc.dram_tensor(
            "desthl", [2 * e_core, NODE_IN], BF16, kind="ExternalInput"
        ).ap()
    else:
        src_d = nc.dram_tensor(
            "src", [e_core, NODE_IN], IN_DT, kind="ExternalInput"
        ).ap()
        dest_d = nc.dram_tensor(
            "dest", [e_core, NODE_IN], IN_DT, kind="ExternalInput"
        ).ap()
    ea_d = nc.dram_tensor("ea", [e_core, EDGE_IN], IN_DT, kind="ExternalInput").ap()
    batch_d = nc.dram_tensor("batch", [e_core], F32, kind="ExternalInput").ap()
    w1_d = nc.dram_tensor("w1", [P, 3, HIDDEN], F32, kind="ExternalInput").ap()
    w1u_d = nc.dram_tensor("w1u", [GLOBAL_IN, HIDDEN], F32, kind="ExternalInput").ap()
    w2_d = nc.dram_tensor("w2", [P, 2, P], F32, kind="ExternalInput").ap()
    b1_d = nc.dram_tensor("b1", [P, 2], F32, kind="ExternalInput").ap()
    b2_d = nc.dram_tensor("b2", [EDGE_OUT, 1], F32, kind="ExternalInput").ap()
    u_d = nc.dram_tensor("u", [B_GLOBAL, GLOBAL_IN], F32, kind="ExternalInput").ap()
    iota_d = nc.dram_tensor("iota", [P, 1], F32, kind="ExternalInput").ap()
    out_d = nc.dram_tensor("out", [EDGE_OUT, e_core], F32, kind="ExternalOutput").ap()

    with tile.TileContext(nc) as tc, ExitStack() as ctx:
        consts = ctx.enter_context(tc.tile_pool(name="consts", bufs=1))
        loads = ctx.enter_context(tc.tile_pool(name="loads", bufs=4))
        acts = ctx.enter_context(tc.tile_pool(name="acts", bufs=4))
        psum = ctx.enter_context(tc.tile_pool(name="psum", bufs=1, space="PSUM"))

        # ---- setup: constants ------------------------------------------
        ident = consts.tile([P, P], F32)
        make_identity(nc, ident[:])
        if IN_DT == F32:
            identt = ident
        else:
            identt = consts.tile([P, P], IN_DT)
            nc.vector.tensor_copy(identt[:], ident[:])

        w1_ld = consts.tile([P, 3, HIDDEN], F32)
        nc.sync.dma_start(w1_ld[:], w1_d)
        w1_sb = consts.tile([P, 3, HIDDEN], MMDT)
        nc.vector.tensor_copy(w1_sb[:], w1_ld[:])
        w1u_sb = consts.tile([GLOBAL_IN, HIDDEN], F32)
        nc.sync.dma_start(w1u_sb[:], w1u_d)
        w2_ld = consts.tile([P, 2, P], F32)
        nc.sync.dma_start(w2_ld[:], w2_d)
        w2_sb = consts.tile([P, 2, P], MMDT)
        nc.vector.tensor_copy(w2_sb[:], w2_ld[:])
        b1_sb = consts.tile([P, 2], F32)
        nc.sync.dma_start(b1_sb[:], b1_d)
        b2_sb = consts.tile([EDGE_OUT, 1], F32)
        nc.sync.dma_start(b2_sb[:], b2_d)
        u_sb = consts.tile([B_GLOBAL, GLOBAL_IN], F32)
        nc.sync.dma_start(u_sb[:], u_d)
        iota_sb = consts.tile([P, 1], F32)
        nc.sync.dma_start(iota_sb[:], iota_d)

        # uW1 = u @ W1u -> [16, 256] landed on partitions 64:80 (col-group
        # packing) so the copy into w1_sb chunk-2 rows 64:80 stays in-lane.
        ps_ut = psum.tile([GLOBAL_IN, B_GLOBAL], F32, tag="ps_eT")
        nc.tensor.transpose(ps_ut[:], u_sb[:], ident[:B_GLOBAL, :B_GLOBAL])
        ut_sb = consts.tile([GLOBAL_IN, B_GLOBAL], F32)
        nc.vector.tensor_copy(ut_sb[:], ps_ut[:])
        ps_uw1 = psum.tile([P, HIDDEN], F32, tag="ps_h0")
        nc.tensor.matmul(ps_uw1[64:80, :], ut_sb[:], w1u_sb[:], start=True, stop=True)
        nc.vector.tensor_copy(w1_sb[64:80, 2, :], ps_uw1[64:80, :])

        # ---- main loop over edge tiles ---------------------------------
        for t in range(n_tiles):
            e0 = t * TILE_E
            esl = slice(e0, e0 + TILE_E)

            if XBAR:
                # xbar transpose: [2*TILE_E, 128] bf16 -> [128, 2*TILE_E]
                # (cols 0:TILE_E = hi, TILE_E:2*TILE_E = lo)
                hsl = slice(2 * e0, 2 * (e0 + TILE_E))
                xs_hl = loads.tile([P, 2 * TILE_E], BF16, tag="xs_hl")
                nc.sync.dma_start_transpose(xs_hl[:], srchl_d[hsl])
                xd_hl = loads.tile([P, 2 * TILE_E], BF16, tag="xd_hl")
                nc.scalar.dma_start_transpose(xd_hl[:], desthl_d[hsl])
                xs = acts.tile([P, TILE_E], MMDT, tag="xs")
                nc.vector.tensor_tensor(
                    xs[:], xs_hl[:, 0:TILE_E], xs_hl[:, TILE_E:],
                    mybir.AluOpType.add,
                )
                xd = acts.tile([P, TILE_E], MMDT, tag="xd")
                nc.vector.tensor_tensor(
                    xd[:], xd_hl[:, 0:TILE_E], xd_hl[:, TILE_E:],
                    mybir.AluOpType.add,
                )
            else:
                a_src = loads.tile([P, SUB, NODE_IN], IN_DT, tag="a_src")
                nc.sync.dma_start(
                    a_src[:], src_d[esl].rearrange("(c p) f -> p c f", p=P)
                )
                a_dest = loads.tile([P, SUB, NODE_IN], IN_DT, tag="a_dest")
                nc.sync.dma_start(
                    a_dest[:], dest_d[esl].rearrange("(c p) f -> p c f", p=P)
                )
                ps_sT = psum.tile([P, TILE_E], IN_DT, tag="ps_sT", bufs=2)
                ps_dT = psum.tile([P, TILE_E], IN_DT, tag="ps_dT", bufs=2)
                for c in range(SUB):
                    csl = slice(c * P, (c + 1) * P)
                    nc.tensor.transpose(ps_sT[:, csl], a_src[:, c, :], identt[:])
                    nc.tensor.transpose(ps_dT[:, csl], a_dest[:, c, :], identt[:])
                xs = acts.tile([P, TILE_E], MMDT, tag="xs")
                nc.vector.tensor_copy(xs[:], ps_sT[:])
                xd = acts.tile([P, TILE_E], MMDT, tag="xd")
                nc.scalar.copy(xd[:], ps_dT[:])

            # edge_attr: edge-major load + PE transpose
            a_ea = loads.tile([P, SUB, EDGE_IN], IN_DT, tag="a_ea")
            nc.sync.dma_start(a_ea[:], ea_d[esl].rearrange("(c p) f -> p c f", p=P))
            ps_eT = psum.tile([EDGE_IN, TILE_E], IN_DT, tag="ps_eT")
            for c in range(SUB):
                nc.tensor.transpose(
                    ps_eT[:, c * P : (c + 1) * P], a_ea[:, c, :], identt[:]
                )

            # chunk-2 rhs tile: rows 0:64 = edge_attr^T, rows 64:80 = one_hot
            chunk2 = acts.tile([80, TILE_E], MMDT, tag="chunk2")
            nc.vector.tensor_copy(chunk2[0:64, :], ps_eT[:])
            b_bcast = loads.tile([80, TILE_E], F32, tag="b_bcast")
            nc.gpsimd.dma_start(
                b_bcast[64:80, :],
                batch_d[esl][None, :].to_broadcast([B_GLOBAL, TILE_E]),
            )
            nc.vector.tensor_scalar(
                chunk2[64:80, :],
                b_bcast[64:80, :],
                iota_sb[64:80, :],
                None,
                mybir.AluOpType.is_equal,
            )

            # layer 1: h^T = W1^T @ x^T -> [256, 512] as 2 psum banks
            ps_h0 = psum.tile([P, TILE_E], F32, tag="ps_h0")
            ps_h1 = psum.tile([P, TILE_E], F32, tag="ps_h1")
            for m, ps_h in enumerate((ps_h0, ps_h1)):
                msl = slice(m * P, (m + 1) * P)
                nc.tensor.matmul(
                    ps_h[:], w1_sb[:, 0, msl], xs[:], start=True, stop=False
                )
                nc.tensor.matmul(
                    ps_h[:], w1_sb[:, 1, msl], xd[:], start=False, stop=False
                )
                nc.tensor.matmul(
                    ps_h[:], w1_sb[0:80, 2, msl], chunk2[:], start=False, stop=True
                )
            # bias + relu: fused on DVE (add then max with 0)
            h = acts.tile([P, 2, TILE_E], MMDT, tag="h")
            nc.vector.tensor_scalar(
                h[:, 0, :], ps_h0[:], b1_sb[:, 0:1], 0.0,
                mybir.AluOpType.add, mybir.AluOpType.max,
            )
            nc.scalar.activation(
                h[:, 1, :], ps_h1[:], mybir.ActivationFunctionType.Relu,
                bias=b1_sb[:, 1:2],
            )

            # layer 2: out^T = W2^T @ h^T -> [64(+pad), 512]
            ps_o = psum.tile([P, TILE_E], F32, tag="ps_o")
            nc.tensor.matmul(
                ps_o[:], w2_sb[:, 0, :], h[:, 0, :], start=True, stop=False
            )
            nc.tensor.matmul(
                ps_o[:], w2_sb[:, 1, :], h[:, 1, :], start=False, stop=True
            )
            # store hidden-major [64, e]; the host unshard transposes the
            # final gather (pure layout, no arithmetic)
            o_sb = acts.tile([EDGE_OUT, TILE_E], F32, tag="o_sb")
            nc.scalar.activation(
                o_sb[:], ps_o[0:EDGE_OUT, :], mybir.ActivationFunctionType.Identity,
                bias=b2_sb[:],
            )
            nc.gpsimd.dma_start(out_d[:, esl], o_sb[:])

    nc.compile()
    return nc


def _hilo(x: np.ndarray, n_tiles: int) -> np.ndarray:
    """[E, F] fp32 -> [2*E, F] bf16 laid out per tile as [hi(512); lo(512)]."""
    hi = x.astype(ml_dtypes.bfloat16)
    lo = (x - hi.astype(np.float32)).astype(ml_dtypes.bfloat16)
    e, f = x.shape
    te = e // n_tiles
    out = np.empty((n_tiles, 2, te, f), dtype=ml_dtypes.bfloat16)
    out[:, 0] = hi.reshape(n_tiles, te, f)
    out[:, 1] = lo.reshape(n_tiles, te, f)
    return np.ascontiguousarray(out.reshape(2 * e, f))


def make_in_maps(inputs: dict, e_core: int = E_CORE, n_cores: int = N_CORES):
    src = np.ascontiguousarray(np.asarray(inputs["src"], dtype=np.float32))
    dest = np.ascontiguousarray(np.asarray(inputs["dest"], dtype=np.float32))
    ea = np.ascontiguousarray(np.asarray(inputs["edge_attr"], dtype=np.float32))
    u = np.ascontiguousarray(np.asarray(inputs["u"], dtype=np.float32))
    batch = np.ascontiguousarray(np.asarray(inputs["batch"]).astype(np.float32))
    W1 = np.asarray(inputs["W1"], dtype=np.float32)
    b1 = np.asarray(inputs["b1"], dtype=np.float32)
    W2 = np.asarray(inputs["W2"], dtype=np.float32)
    b2 = np.asarray(inputs["b2"], dtype=np.float32)

    # host-side weight layout shuffles (no arithmetic)
    w1_r = np.zeros((P, 3, HIDDEN), dtype=np.float32)
    w1_r[:, 0, :] = W1[0:128]
    w1_r[:, 1, :] = W1[128:256]
    w1_r[0:64, 2, :] = W1[256:320]
    w1u = np.ascontiguousarray(W1[320:384])
    w2_r = np.zeros((P, 2, P), dtype=np.float32)
    w2_r[:, :, :EDGE_OUT] = W2.reshape(2, P, EDGE_OUT).transpose(1, 0, 2)
    b1_r = np.ascontiguousarray(b1.reshape(2, P).T)
    b2_r = np.ascontiguousarray(b2.reshape(EDGE_OUT, 1))
    iota = np.zeros((P, 1), dtype=np.float32)
    iota[64:80, 0] = np.arange(16)

    n_tiles = e_core // TILE_E
    in_maps = []
    for c in range(n_cores):
        esl = slice(c * e_core, (c + 1) * e_core)
        m = {
            "ea": ea[esl].astype(NPDT) if TWO_BYTE else ea[esl],
            "batch": batch[esl],
            "w1": w1_r,
            "w1u": w1u,
            "w2": w2_r,
            "b1": b1_r,
            "b2": b2_r,
            "u": u,
            "iota": iota,
        }
        if XBAR:
            m["srchl"] = _hilo(src[esl], n_tiles)
            m["desthl"] = _hilo(dest[esl], n_tiles)
        elif TWO_BYTE:
            m["src"] = src[esl].astype(NPDT)
            m["dest"] = dest[esl].astype(NPDT)
        else:
            m["src"] = src[esl]
            m["dest"] = dest[esl]
        in_maps.append(m)
    return in_maps


_CACHED_NC = None
last_exec_time_ns = None
last_profile_json = None


def kernel(**inputs) -> np.ndarray:
    global _CACHED_NC, last_exec_time_ns, last_profile_json
    if _CACHED_NC is None:
        _CACHED_NC = build_program()
    nc = _CACHED_NC
    in_maps = make_in_maps(inputs)
    trace = os.environ.get("KERNEL_TRACE", "0") == "1"
    res = run_bass_kernel_spmd(
        nc, in_maps, core_ids=list(range(N_CORES)), trace=trace
    )
    last_exec_time_ns = res.exec_time_ns
    last_profile_json = res.profile_json
    out = np.concatenate(
        [res.results[c]["out"].T for c in range(N_CORES)], axis=0
    )
    return np.ascontiguousarray(out)
